# revision 22
# baseline (speedup 1.0000x reference)
"""EndPointAggregator Trainium2 kernel — PE one-hot expansion version.

out[j] = concat(table[starts[j]], table[ends[j]], tanh((ends[j]-starts[j]) @ w.T + b))

Strategy (8 NeuronCores, data-parallel over spans):
  - each core owns 25000 spans, padded to NPAD=25088; per side (start/end)
    the spans are sorted by table row (host-chosen slot order; `assemble`
    unpermutes), so each 128-span chunk touches <=32 distinct table rows
  - SWDGE dma_gather pulls only those distinct rows (bf16 table, 1536B
    rows) from HBM: ~19 MB/core instead of the 154 MB a full per-span
    gather would read
  - TensorE expands windows into per-span rows: for each chunk,
    psum[128 spans, 768] = onehotT[32, 128].T @ window[32, 768] (bf16 in,
    f32 psum). Window w of a 16-chunk gather group lands at partitions
    32*(w%4), rank w//4, so K=32 matmuls rotate row strips via
    tile_position.
  - ACT copies psum[:, 0:512] and DVE psum[:, 512:768] into an SBUF
    staging tile (4 chunks = 512 output rows), HWDGE streams it to HBM —
    the only large HBM stream left (~154 MB/core write).
  - dist_emb = tanh(w*(e-s)+b) computed once for the whole core on DVE/ACT
  - three device outputs (outS/outE/outD); host reassembles [200000, 1538]

Values pass through bf16 once (table rows), so outS/outE are bf16-rounded
f32: rel err ~2e-3 against the f32 reference (harness gate is 2e-2).
"""

import numpy as np
import ml_dtypes

import concourse.bacc as bacc
import concourse.bass as bass
import concourse.mybir as mybir
import concourse.tile as tile
from concourse.bass_utils import run_bass_kernel_spmd

N_CORES = 8
SEQ_LEN = 4096
DIM = 768
N_SPANS = 200000

N_PER_CORE = N_SPANS // N_CORES  # 25000
CH = 128                          # spans per chunk (psum partition dim)
NCHK = 196                        # chunks per side per core
NPAD = NCHK * CH                  # 25088
PERP = NPAD // 128                # dist layout cols (196)
W = 32                            # window rows gathered per chunk
GRP = 16                          # chunks per dma_gather instruction
NG_FULL = NCHK // GRP             # 12 full gather groups
TAILC = NCHK - NG_FULL * GRP      # 4 chunks in the tail group
IDXCOLS = NG_FULL * (GRP * W // 16) + (TAILC * W // 16)  # 392
NTILE = NCHK // 4                 # 49 4-chunk output tiles per side
OHCOLS = NTILE * CH               # 6272 onehot cols per strip

F32 = mybir.dt.float32
F16 = mybir.dt.float16
FP8 = mybir.dt.float8e4
I32 = mybir.dt.int32
I16 = mybir.dt.int16


def _device_row_perm():
    """P[r] = sorted position held by device output row r.

    Full tiles: r = 1024*T + 8*m + j  <->  sorted pos 1024*T + 128*j + m.
    Tail (last 512 rows): r = 24576 + 4*m + j  <->  pos 24576 + 128*j + m.
    """
    r = np.arange(NPAD)
    nfull = (NCHK // 8) * 1024
    P = np.empty(NPAD, np.int64)
    rf = r[:nfull]
    P[:nfull] = (rf // 1024) * 1024 + (rf % 8) * 128 + (rf % 1024) // 8
    rt = r[nfull:] - nfull
    P[nfull:] = nfull + (rt % 4) * 128 + rt // 4
    return P


_DEV_PERM = _device_row_perm()


def build_module(trace_sim=False):
    """Build the per-core Bass module (same NEFF on all 8 cores)."""
    nc = bacc.Bacc(
        "TRN2",
        target_bir_lowering=False,
        debug=False,
        num_devices=N_CORES,
    )
    table = nc.dram_tensor("table", [SEQ_LEN, DIM], F16, kind="ExternalInput").ap()
    idx_s = nc.dram_tensor("idx_s", [128, IDXCOLS], I16, kind="ExternalInput").ap()
    idx_e = nc.dram_tensor("idx_e", [128, IDXCOLS], I16, kind="ExternalInput").ap()
    oh_s = nc.dram_tensor("oh_s", [128, OHCOLS], FP8, kind="ExternalInput").ap()
    oh_e = nc.dram_tensor("oh_e", [128, OHCOLS], FP8, kind="ExternalInput").ap()
    s_c = nc.dram_tensor("s_c", [128, PERP], I32, kind="ExternalInput").ap()
    e_c = nc.dram_tensor("e_c", [128, PERP], I32, kind="ExternalInput").ap()
    wb = nc.dram_tensor("wb", [1, 4], F32, kind="ExternalInput").ap()
    # outputs are fp16: the values are fp16-exact anyway (table went through
    # fp16), and halving the 154 MB/core write stream is the dominant win;
    # the host casts back to f32 during assemble.
    outS = nc.dram_tensor("outS", [NPAD, DIM], F16, kind="ExternalOutput").ap()
    outE = nc.dram_tensor("outE", [NPAD, DIM], F16, kind="ExternalOutput").ap()
    outD = nc.dram_tensor("outD", [128, PERP * 2], F32, kind="ExternalOutput").ap()

    # 8-chunk m-major view: row = 1024*T + 8*m + j  ->  outX_v[T] is [m, j, d]
    # (each partition's 8 rows are contiguous -> 24KB DMA descriptors)
    nfull = (NCHK // 8) * 1024  # rows covered by full 8-chunk tiles (24576)
    outS_v = outS[:nfull].rearrange("(t m j) d -> t m j d", j=8, m=128)
    outE_v = outE[:nfull].rearrange("(t m j) d -> t m j d", j=8, m=128)
    # tail (4 chunks, 512 rows): row = nfull + 4*m + j
    outS_t = outS[nfull:].rearrange("(m j) d -> m j d", j=4)
    outE_t = outE[nfull:].rearrange("(m j) d -> m j d", j=4)

    with tile.TileContext(nc, trace_sim=trace_sim) as tc:
        with (
            tc.tile_pool(name="const", bufs=1) as cpool,
            tc.tile_pool(name="win", bufs=6) as wpool,
            tc.tile_pool(name="stage", bufs=4) as spool,
            tc.psum_pool(name="ps", bufs=4) as ppool,
        ):
            # ---- resident inputs ----
            # idx on gpsimd (feeds the gathers there), oh on sync, dist on
            # scalar: three queues load in parallel at startup.
            idx_s_t = cpool.tile([128, IDXCOLS], I16)
            idx_e_t = cpool.tile([128, IDXCOLS], I16)
            nc.gpsimd.dma_start(out=idx_s_t[:], in_=idx_s)
            nc.gpsimd.dma_start(out=idx_e_t[:], in_=idx_e)
            oh_s_t = cpool.tile([128, OHCOLS], FP8)
            oh_e_t = cpool.tile([128, OHCOLS], FP8)
            nc.sync.dma_start(out=oh_s_t[:], in_=oh_s)
            nc.sync.dma_start(out=oh_e_t[:], in_=oh_e)

            # dist_emb inputs loaded early (tiny); the compute is scheduled at
            # the S->E boundary where the pipeline has a natural bubble.
            s_t = cpool.tile([128, PERP], I32)
            e_t = cpool.tile([128, PERP], I32)
            nc.scalar.dma_start(out=s_t[:], in_=s_c)
            nc.scalar.dma_start(out=e_t[:], in_=e_c)
            wb_t = cpool.tile([128, 4], F32, tag="wb_in")
            nc.scalar.dma_start(out=wb_t[:1, :], in_=wb)

            # ---- main loop: gather windows, PE-expand, copy, write out ----
            for side in range(2):
                if side == 1:
                    # ---- dist_emb chain in the inter-side bubble ----
                    wb_bc = cpool.tile([128, 4], F32, tag="wb_bc")
                    nc.gpsimd.partition_broadcast(wb_bc[:], wb_t[:1, :])
                    d_i = cpool.tile([128, PERP], I32)
                    nc.vector.tensor_tensor(
                        out=d_i[:], in0=e_t[:], in1=s_t[:],
                        op=mybir.AluOpType.subtract,
                    )
                    d_f = cpool.tile([128, PERP], F32)
                    nc.vector.tensor_copy(out=d_f[:], in_=d_i[:])
                    dist = cpool.tile([128, PERP, 2], F32)
                    nc.scalar.activation(
                        dist[:, :, 0],
                        d_f[:],
                        mybir.ActivationFunctionType.Tanh,
                        bias=wb_bc[:, 2:3],
                        scale=wb_bc[:, 0:1],
                    )
                    nc.scalar.activation(
                        dist[:, :, 1],
                        d_f[:],
                        mybir.ActivationFunctionType.Tanh,
                        bias=wb_bc[:, 3:4],
                        scale=wb_bc[:, 1:2],
                    )
                    nc.scalar.dma_start(
                        out=outD, in_=dist[:].rearrange("p c two -> p (c two)")
                    )
                idx_t = (idx_s_t, idx_e_t)[side]
                oh_t = (oh_s_t, oh_e_t)[side]
                outv = (outS_v, outE_v)[side]
                outtail = (outS_t, outE_t)[side]
                for g in range(NG_FULL + 1):
                    nch = GRP if g < NG_FULL else TAILC
                    nidx = nch * W
                    col0 = g * (GRP * W // 16)
                    split0 = side == 0 and g == 0  # finer pipeline fill
                    if split0:
                        wtA = wpool.tile([128, 2, DIM], F16, tag="win2")
                        wtB = wpool.tile([128, 2, DIM], F16, tag="win2")
                        nc.gpsimd.dma_gather(
                            wtA[:], table, idx_t[:, 0:16], 256, 256, DIM,
                            single_packet=False,
                        )
                        nc.gpsimd.dma_gather(
                            wtB[:], table, idx_t[:, 16:32], 256, 256, DIM,
                            single_packet=False,
                        )
                    else:
                        wtile = wpool.tile([128, 4, DIM], F16, tag="win")
                        nc.gpsimd.dma_gather(
                            wtile[:, : nch // 4, :],
                            table,
                            idx_t[:, col0 : col0 + nidx // 16],
                            nidx,
                            nidx,
                            DIM,
                            single_packet=False,
                        )
                    for h in range(max(1, nch // 8)):  # 8-chunk halves of the group
                        stage = spool.tile([128, 8, 2, 384], F16, tag="stage")
                        nj = 8 if nch >= 8 else 4  # chunks staged this round
                        for j in range(nj):  # one 2-bank psum tile per chunk
                            ps2 = ppool.tile([128, 2, 512], F32, tag="ps2")
                            qq, s = j // 4, j % 4
                            q = 2 * h + qq
                            t = 4 * g + q  # strip ordinal (onehot col block)
                            lhsT = oh_t[
                                32 * s : 32 * (s + 1), t * CH : (t + 1) * CH
                            ]
                            if split0:
                                wt = wtA if h == 0 else wtB
                                rhs = wt[32 * s : 32 * (s + 1), q - 2 * h, :]
                            else:
                                rhs = wtile[32 * s : 32 * (s + 1), q, :]
                            nc.tensor.matmul(
                                ps2[:, 0, 0:384], lhsT, rhs[:, 0:384],
                                tile_position=(32 * s, 0),
                            )
                            nc.tensor.matmul(
                                ps2[:, 1, 0:384], lhsT, rhs[:, 384:768],
                                tile_position=(32 * s, 0),
                            )
                            eng = nc.scalar.copy if j % 2 == 0 else (
                                lambda out, in_: nc.vector.tensor_copy(
                                    out=out, in_=in_
                                )
                            )
                            eng(out=stage[:, j], in_=ps2[:, :, 0:384])
                            if split0 and h == 0 and j % 2 == 1:
                                # early 2-chunk writeouts while the pipe fills
                                nc.sync.dma_start(
                                    out=outv[2 * g + h][:, j - 1 : j + 1, :],
                                    in_=stage[:, j - 1 : j + 1].rearrange(
                                        "m a k n -> m a (k n)"
                                    ),
                                )
                        if split0 and h == 0:
                            pass  # already written in 2-chunk pieces
                        elif nch >= 8:
                            T = 2 * g + h
                            nc.sync.dma_start(
                                out=outv[T],
                                in_=stage[:].rearrange("m a k n -> m a (k n)"),
                            )
                        else:
                            nc.sync.dma_start(
                                out=outtail,
                                in_=stage[:, 0:4].rearrange("m a k n -> m a (k n)"),
                            )

    nc.compile()
    return nc


def _wrap_idx(v):
    """idx i -> (partition i%16, col i//16), replicated x8 -> [128, len//16]."""
    w = v.reshape(-1, 16).T
    return np.tile(w, (8, 1)).copy()


def _prep_side(vals):
    """vals: [N_PER_CORE] int span endpoints for one side of one core.

    Returns (idx_wrapped [128, IDXCOLS] i16, onehot [128, OHCOLS] bf16,
             order [NPAD] so that device_row[i] = original_slot[order[i]])."""
    v = np.zeros(NPAD, np.int32)
    v[:N_PER_CORE] = vals
    order = np.argsort(v, kind="stable")
    v = v[order].reshape(NCHK, CH)

    newrow = np.ones((NCHK, CH), bool)
    newrow[:, 1:] = v[:, 1:] != v[:, :-1]
    j = np.cumsum(newrow, axis=1) - 1  # position of each span's row in window
    d = j[:, -1] + 1
    assert d.max() <= W, f"chunk with {d.max()} distinct rows exceeds W={W}"

    win = np.repeat(v[:, -1:], W, axis=1).astype(np.int32)
    ci = np.repeat(np.arange(NCHK), CH)
    win[ci, j.ravel()] = v.ravel()
    win = win.astype(np.int16)

    oh = np.zeros((NCHK, W, CH), ml_dtypes.float8_e4m3)
    oh[ci, j.ravel(), np.tile(np.arange(CH), NCHK)] = 1.0

    # gather idx stream: full groups of 16 chunks (512 idxs), tail of 4 (128)
    cols = [
        _wrap_idx(win[g * GRP : (g + 1) * GRP].ravel()) for g in range(NG_FULL)
    ]
    cols.append(_wrap_idx(win[NG_FULL * GRP :].ravel()))
    idx = np.concatenate(cols, axis=1)
    assert idx.shape == (128, IDXCOLS)

    # onehot resident layout: OH[32*s + k, o*128 + m] = oh[4*o + s, k, m]
    ohr = (
        oh.reshape(NTILE, 4, W, CH)
        .transpose(1, 2, 0, 3)
        .reshape(128, OHCOLS)
        .copy()
    )
    return idx, ohr, order[_DEV_PERM]


def _prep_core_inputs(starts, ends, dist_w, dist_b, table_bf16):
    idx_s, oh_s, order_s = _prep_side(starts)
    idx_e, oh_e, order_e = _prep_side(ends)

    sw = np.zeros(NPAD, np.int32)
    ew = np.zeros(NPAD, np.int32)
    sw[:N_PER_CORE] = starts.astype(np.int32)
    ew[:N_PER_CORE] = ends.astype(np.int32)

    wbv = np.array(
        [[dist_w[0, 0], dist_w[1, 0], dist_b[0], dist_b[1]]], np.float32
    )
    return (
        {
            "table": table_bf16,
            "idx_s": idx_s,
            "idx_e": idx_e,
            "oh_s": oh_s,
            "oh_e": oh_e,
            "s_c": sw.reshape(128, PERP),
            "e_c": ew.reshape(128, PERP),
            "wb": wbv,
        },
        order_s,
        order_e,
    )


_module_cache = {}


def get_module():
    if "nc" not in _module_cache:
        _module_cache["nc"] = build_module()
    return _module_cache["nc"]


def make_in_maps(sentence_embeddings, sentence_spans, dist_w, dist_b):
    table_f32 = np.ascontiguousarray(np.asarray(sentence_embeddings, np.float32))
    table_bf16 = table_f32.astype(np.float16)
    spans = np.asarray(sentence_spans)
    dist_w = np.asarray(dist_w, np.float32)
    dist_b = np.asarray(dist_b, np.float32)
    starts = spans[:, 0]
    ends = spans[:, 1]
    in_maps = []
    orders = []
    for c in range(N_CORES):
        sl = slice(c * N_PER_CORE, (c + 1) * N_PER_CORE)
        m, os_, oe_ = _prep_core_inputs(
            starts[sl], ends[sl], dist_w, dist_b, table_bf16
        )
        in_maps.append(m)
        orders.append((os_, oe_))
    return in_maps, orders


def run_spmd(in_maps, **kw):
    return run_bass_kernel_spmd(
        get_module(), in_maps, core_ids=list(range(N_CORES)), **kw
    )


def assemble(results, orders):
    out = np.empty((N_SPANS, 2 * DIM + 2), np.float32)
    tmp = np.empty((NPAD, DIM), np.float32)
    for c, r in enumerate(results):
        order_s, order_e = orders[c]
        sl = slice(c * N_PER_CORE, (c + 1) * N_PER_CORE)
        tmp[order_s] = np.asarray(r["outS"]).astype(np.float32)
        out[sl, :DIM] = tmp[:N_PER_CORE]
        tmp[order_e] = np.asarray(r["outE"]).astype(np.float32)
        out[sl, DIM : 2 * DIM] = tmp[:N_PER_CORE]
        out[sl, 2 * DIM :] = r["outD"].reshape(NPAD, 2)[:N_PER_CORE]
    return out


def kernel(sentence_embeddings, sentence_spans, dist_w, dist_b):
    in_maps, orders = make_in_maps(sentence_embeddings, sentence_spans, dist_w, dist_b)
    res = run_spmd(in_maps)
    return assemble(res.results, orders)


# revision 23
# speedup vs baseline: 1.0537x; 1.0537x over previous
"""EndPointAggregator Trainium2 kernel — PE one-hot expansion version.

out[j] = concat(table[starts[j]], table[ends[j]], tanh((ends[j]-starts[j]) @ w.T + b))

Strategy (8 NeuronCores, data-parallel over spans):
  - each core owns 25000 spans, padded to NPAD=25088; per side (start/end)
    the spans are sorted by table row (host-chosen slot order; `assemble`
    unpermutes), so each 128-span chunk touches <=32 distinct table rows
  - SWDGE dma_gather pulls only those distinct rows (bf16 table, 1536B
    rows) from HBM: ~19 MB/core instead of the 154 MB a full per-span
    gather would read
  - TensorE expands windows into per-span rows: for each chunk,
    psum[128 spans, 768] = onehotT[32, 128].T @ window[32, 768] (bf16 in,
    f32 psum). Window w of a 16-chunk gather group lands at partitions
    32*(w%4), rank w//4, so K=32 matmuls rotate row strips via
    tile_position.
  - ACT copies psum[:, 0:512] and DVE psum[:, 512:768] into an SBUF
    staging tile (4 chunks = 512 output rows), HWDGE streams it to HBM —
    the only large HBM stream left (~154 MB/core write).
  - dist_emb = tanh(w*(e-s)+b) computed once for the whole core on DVE/ACT
  - three device outputs (outS/outE/outD); host reassembles [200000, 1538]

Values pass through bf16 once (table rows), so outS/outE are bf16-rounded
f32: rel err ~2e-3 against the f32 reference (harness gate is 2e-2).
"""

import numpy as np
import ml_dtypes

import concourse.bacc as bacc
import concourse.bass as bass
import concourse.mybir as mybir
import concourse.tile as tile
from concourse.bass_utils import run_bass_kernel_spmd

N_CORES = 8
SEQ_LEN = 4096
DIM = 768
N_SPANS = 200000

N_PER_CORE = N_SPANS // N_CORES  # 25000
CH = 128                          # spans per chunk (psum partition dim)
NCHK = 196                        # chunks per side per core
NPAD = NCHK * CH                  # 25088
PERP = NPAD // 128                # dist layout cols (196)
W = 32                            # window rows gathered per chunk
GRP = 16                          # chunks per dma_gather instruction
NG_FULL = NCHK // GRP             # 12 full gather groups
TAILC = NCHK - NG_FULL * GRP      # 4 chunks in the tail group
IDXCOLS = NG_FULL * (GRP * W // 16) + (TAILC * W // 16)  # 392
NTILE = NCHK // 4                 # 49 4-chunk output tiles per side
OHCOLS = NTILE * CH               # 6272 onehot cols per strip

F32 = mybir.dt.float32
F16 = mybir.dt.float16
FP8 = mybir.dt.float8e4
I32 = mybir.dt.int32
I16 = mybir.dt.int16


def _device_row_perm():
    """P[r] = sorted position held by device output row r.

    Full tiles: r = 1024*T + 8*m + j  <->  sorted pos 1024*T + 128*j + m.
    Tail (last 512 rows): r = 24576 + 4*m + j  <->  pos 24576 + 128*j + m.
    """
    r = np.arange(NPAD)
    nfull = (NCHK // 8) * 1024
    P = np.empty(NPAD, np.int64)
    rf = r[:nfull]
    P[:nfull] = (rf // 1024) * 1024 + (rf % 8) * 128 + (rf % 1024) // 8
    rt = r[nfull:] - nfull
    P[nfull:] = nfull + (rt % 4) * 128 + rt // 4
    return P


_DEV_PERM = _device_row_perm()


def build_module(trace_sim=False):
    """Build the per-core Bass module (same NEFF on all 8 cores)."""
    nc = bacc.Bacc(
        "TRN2",
        target_bir_lowering=False,
        debug=False,
        num_devices=N_CORES,
    )
    table = nc.dram_tensor("table", [SEQ_LEN, DIM], F16, kind="ExternalInput").ap()
    idx_s = nc.dram_tensor("idx_s", [128, IDXCOLS], I16, kind="ExternalInput").ap()
    idx_e = nc.dram_tensor("idx_e", [128, IDXCOLS], I16, kind="ExternalInput").ap()
    oh_s = nc.dram_tensor("oh_s", [128, OHCOLS], FP8, kind="ExternalInput").ap()
    oh_e = nc.dram_tensor("oh_e", [128, OHCOLS], FP8, kind="ExternalInput").ap()
    s_c = nc.dram_tensor("s_c", [128, PERP], I32, kind="ExternalInput").ap()
    e_c = nc.dram_tensor("e_c", [128, PERP], I32, kind="ExternalInput").ap()
    wb = nc.dram_tensor("wb", [1, 4], F32, kind="ExternalInput").ap()
    # outputs are fp16: the values are fp16-exact anyway (table went through
    # fp16), and halving the 154 MB/core write stream is the dominant win;
    # the host casts back to f32 during assemble.
    outS = nc.dram_tensor("outS", [NPAD, DIM], F16, kind="ExternalOutput").ap()
    outE = nc.dram_tensor("outE", [NPAD, DIM], F16, kind="ExternalOutput").ap()
    outD = nc.dram_tensor("outD", [128, PERP * 2], F32, kind="ExternalOutput").ap()

    # 8-chunk m-major view: row = 1024*T + 8*m + j  ->  outX_v[T] is [m, j, d]
    # (each partition's 8 rows are contiguous -> 24KB DMA descriptors)
    nfull = (NCHK // 8) * 1024  # rows covered by full 8-chunk tiles (24576)
    outS_v = outS[:nfull].rearrange("(t m j) d -> t m j d", j=8, m=128)
    outE_v = outE[:nfull].rearrange("(t m j) d -> t m j d", j=8, m=128)
    # tail (4 chunks, 512 rows): row = nfull + 4*m + j
    outS_t = outS[nfull:].rearrange("(m j) d -> m j d", j=4)
    outE_t = outE[nfull:].rearrange("(m j) d -> m j d", j=4)

    with tile.TileContext(nc, trace_sim=trace_sim) as tc:
        with (
            tc.tile_pool(name="const", bufs=1) as cpool,
            tc.tile_pool(name="win", bufs=6) as wpool,
            tc.tile_pool(name="stage", bufs=6) as spool,
            tc.psum_pool(name="ps", bufs=4) as ppool,
        ):
            # ---- resident inputs ----
            # idx on gpsimd (feeds the gathers there), oh on sync, dist on
            # scalar: three queues load in parallel at startup.
            idx_s_t = cpool.tile([128, IDXCOLS], I16)
            idx_e_t = cpool.tile([128, IDXCOLS], I16)
            nc.gpsimd.dma_start(out=idx_s_t[:], in_=idx_s)
            nc.gpsimd.dma_start(out=idx_e_t[:], in_=idx_e)
            oh_s_t = cpool.tile([128, OHCOLS], FP8)
            oh_e_t = cpool.tile([128, OHCOLS], FP8)
            nc.sync.dma_start(out=oh_s_t[:], in_=oh_s)
            nc.sync.dma_start(out=oh_e_t[:], in_=oh_e)

            # dist_emb inputs loaded early (tiny); the compute is scheduled at
            # the S->E boundary where the pipeline has a natural bubble.
            s_t = cpool.tile([128, PERP], I32)
            e_t = cpool.tile([128, PERP], I32)
            nc.scalar.dma_start(out=s_t[:], in_=s_c)
            nc.scalar.dma_start(out=e_t[:], in_=e_c)
            wb_t = cpool.tile([128, 4], F32, tag="wb_in")
            nc.scalar.dma_start(out=wb_t[:1, :], in_=wb)

            # ---- main loop: gather windows, PE-expand, copy, write out ----
            for side in range(2):
                if side == 1:
                    # ---- dist_emb chain in the inter-side bubble ----
                    wb_bc = cpool.tile([128, 4], F32, tag="wb_bc")
                    nc.gpsimd.partition_broadcast(wb_bc[:], wb_t[:1, :])
                    d_i = cpool.tile([128, PERP], I32)
                    nc.vector.tensor_tensor(
                        out=d_i[:], in0=e_t[:], in1=s_t[:],
                        op=mybir.AluOpType.subtract,
                    )
                    d_f = cpool.tile([128, PERP], F32)
                    nc.vector.tensor_copy(out=d_f[:], in_=d_i[:])
                    dist = cpool.tile([128, PERP, 2], F32)
                    nc.scalar.activation(
                        dist[:, :, 0],
                        d_f[:],
                        mybir.ActivationFunctionType.Tanh,
                        bias=wb_bc[:, 2:3],
                        scale=wb_bc[:, 0:1],
                    )
                    nc.scalar.activation(
                        dist[:, :, 1],
                        d_f[:],
                        mybir.ActivationFunctionType.Tanh,
                        bias=wb_bc[:, 3:4],
                        scale=wb_bc[:, 1:2],
                    )
                    nc.scalar.dma_start(
                        out=outD, in_=dist[:].rearrange("p c two -> p (c two)")
                    )
                idx_t = (idx_s_t, idx_e_t)[side]
                oh_t = (oh_s_t, oh_e_t)[side]
                outv = (outS_v, outE_v)[side]
                outtail = (outS_t, outE_t)[side]
                for g in range(NG_FULL + 1):
                    nch = GRP if g < NG_FULL else TAILC
                    nidx = nch * W
                    col0 = g * (GRP * W // 16)
                    split0 = side == 0 and g == 0  # finer pipeline fill
                    if split0:
                        wtA = wpool.tile([128, 2, DIM], F16, tag="win2")
                        wtB = wpool.tile([128, 2, DIM], F16, tag="win2")
                        nc.gpsimd.dma_gather(
                            wtA[:], table, idx_t[:, 0:16], 256, 256, DIM,
                            single_packet=False,
                        )
                        nc.gpsimd.dma_gather(
                            wtB[:], table, idx_t[:, 16:32], 256, 256, DIM,
                            single_packet=False,
                        )
                    else:
                        wtile = wpool.tile([128, 4, DIM], F16, tag="win")
                        nc.gpsimd.dma_gather(
                            wtile[:, : nch // 4, :],
                            table,
                            idx_t[:, col0 : col0 + nidx // 16],
                            nidx,
                            nidx,
                            DIM,
                            single_packet=False,
                        )
                    for h in range(max(1, nch // 8)):  # 8-chunk halves of the group
                        stage = spool.tile([128, 8, 2, 384], F16, tag="stage")
                        nj = 8 if nch >= 8 else 4  # chunks staged this round
                        for j in range(nj):  # one 2-bank psum tile per chunk
                            ps2 = ppool.tile([128, 2, 512], F32, tag="ps2")
                            qq, s = j // 4, j % 4
                            q = 2 * h + qq
                            t = 4 * g + q  # strip ordinal (onehot col block)
                            lhsT = oh_t[
                                32 * s : 32 * (s + 1), t * CH : (t + 1) * CH
                            ]
                            if split0:
                                wt = wtA if h == 0 else wtB
                                rhs = wt[32 * s : 32 * (s + 1), q - 2 * h, :]
                            else:
                                rhs = wtile[32 * s : 32 * (s + 1), q, :]
                            nc.tensor.matmul(
                                ps2[:, 0, 0:384], lhsT, rhs[:, 0:384],
                                tile_position=(32 * s, 0),
                            )
                            nc.tensor.matmul(
                                ps2[:, 1, 0:384], lhsT, rhs[:, 384:768],
                                tile_position=(32 * s, 0),
                            )
                            eng = nc.scalar.copy if j % 2 == 0 else (
                                lambda out, in_: nc.vector.tensor_copy(
                                    out=out, in_=in_
                                )
                            )
                            eng(out=stage[:, j], in_=ps2[:, :, 0:384])
                            if split0 and h == 0 and j % 2 == 1:
                                # early 2-chunk writeouts while the pipe fills
                                nc.sync.dma_start(
                                    out=outv[2 * g + h][:, j - 1 : j + 1, :],
                                    in_=stage[:, j - 1 : j + 1].rearrange(
                                        "m a k n -> m a (k n)"
                                    ),
                                )
                        if split0 and h == 0:
                            pass  # already written in 2-chunk pieces
                        elif nch >= 8:
                            T = 2 * g + h
                            nc.sync.dma_start(
                                out=outv[T],
                                in_=stage[:].rearrange("m a k n -> m a (k n)"),
                            )
                        else:
                            nc.sync.dma_start(
                                out=outtail,
                                in_=stage[:, 0:4].rearrange("m a k n -> m a (k n)"),
                            )

    nc.compile()
    return nc


def _wrap_idx(v):
    """idx i -> (partition i%16, col i//16), replicated x8 -> [128, len//16]."""
    w = v.reshape(-1, 16).T
    return np.tile(w, (8, 1)).copy()


def _prep_side(vals):
    """vals: [N_PER_CORE] int span endpoints for one side of one core.

    Returns (idx_wrapped [128, IDXCOLS] i16, onehot [128, OHCOLS] bf16,
             order [NPAD] so that device_row[i] = original_slot[order[i]])."""
    v = np.zeros(NPAD, np.int32)
    v[:N_PER_CORE] = vals
    order = np.argsort(v, kind="stable")
    v = v[order].reshape(NCHK, CH)

    newrow = np.ones((NCHK, CH), bool)
    newrow[:, 1:] = v[:, 1:] != v[:, :-1]
    j = np.cumsum(newrow, axis=1) - 1  # position of each span's row in window
    d = j[:, -1] + 1
    assert d.max() <= W, f"chunk with {d.max()} distinct rows exceeds W={W}"

    win = np.repeat(v[:, -1:], W, axis=1).astype(np.int32)
    ci = np.repeat(np.arange(NCHK), CH)
    win[ci, j.ravel()] = v.ravel()
    win = win.astype(np.int16)

    oh = np.zeros((NCHK, W, CH), ml_dtypes.float8_e4m3)
    oh[ci, j.ravel(), np.tile(np.arange(CH), NCHK)] = 1.0

    # gather idx stream: full groups of 16 chunks (512 idxs), tail of 4 (128)
    cols = [
        _wrap_idx(win[g * GRP : (g + 1) * GRP].ravel()) for g in range(NG_FULL)
    ]
    cols.append(_wrap_idx(win[NG_FULL * GRP :].ravel()))
    idx = np.concatenate(cols, axis=1)
    assert idx.shape == (128, IDXCOLS)

    # onehot resident layout: OH[32*s + k, o*128 + m] = oh[4*o + s, k, m]
    ohr = (
        oh.reshape(NTILE, 4, W, CH)
        .transpose(1, 2, 0, 3)
        .reshape(128, OHCOLS)
        .copy()
    )
    return idx, ohr, order[_DEV_PERM]


def _prep_core_inputs(starts, ends, dist_w, dist_b, table_bf16):
    idx_s, oh_s, order_s = _prep_side(starts)
    idx_e, oh_e, order_e = _prep_side(ends)

    sw = np.zeros(NPAD, np.int32)
    ew = np.zeros(NPAD, np.int32)
    sw[:N_PER_CORE] = starts.astype(np.int32)
    ew[:N_PER_CORE] = ends.astype(np.int32)

    wbv = np.array(
        [[dist_w[0, 0], dist_w[1, 0], dist_b[0], dist_b[1]]], np.float32
    )
    return (
        {
            "table": table_bf16,
            "idx_s": idx_s,
            "idx_e": idx_e,
            "oh_s": oh_s,
            "oh_e": oh_e,
            "s_c": sw.reshape(128, PERP),
            "e_c": ew.reshape(128, PERP),
            "wb": wbv,
        },
        order_s,
        order_e,
    )


_module_cache = {}


def get_module():
    if "nc" not in _module_cache:
        _module_cache["nc"] = build_module()
    return _module_cache["nc"]


def make_in_maps(sentence_embeddings, sentence_spans, dist_w, dist_b):
    table_f32 = np.ascontiguousarray(np.asarray(sentence_embeddings, np.float32))
    table_bf16 = table_f32.astype(np.float16)
    spans = np.asarray(sentence_spans)
    dist_w = np.asarray(dist_w, np.float32)
    dist_b = np.asarray(dist_b, np.float32)
    starts = spans[:, 0]
    ends = spans[:, 1]
    in_maps = []
    orders = []
    for c in range(N_CORES):
        sl = slice(c * N_PER_CORE, (c + 1) * N_PER_CORE)
        m, os_, oe_ = _prep_core_inputs(
            starts[sl], ends[sl], dist_w, dist_b, table_bf16
        )
        in_maps.append(m)
        orders.append((os_, oe_))
    return in_maps, orders


def run_spmd(in_maps, **kw):
    return run_bass_kernel_spmd(
        get_module(), in_maps, core_ids=list(range(N_CORES)), **kw
    )


def assemble(results, orders):
    out = np.empty((N_SPANS, 2 * DIM + 2), np.float32)
    tmp = np.empty((NPAD, DIM), np.float32)
    for c, r in enumerate(results):
        order_s, order_e = orders[c]
        sl = slice(c * N_PER_CORE, (c + 1) * N_PER_CORE)
        tmp[order_s] = np.asarray(r["outS"]).astype(np.float32)
        out[sl, :DIM] = tmp[:N_PER_CORE]
        tmp[order_e] = np.asarray(r["outE"]).astype(np.float32)
        out[sl, DIM : 2 * DIM] = tmp[:N_PER_CORE]
        out[sl, 2 * DIM :] = r["outD"].reshape(NPAD, 2)[:N_PER_CORE]
    return out


def kernel(sentence_embeddings, sentence_spans, dist_w, dist_b):
    in_maps, orders = make_in_maps(sentence_embeddings, sentence_spans, dist_w, dist_b)
    res = run_spmd(in_maps)
    return assemble(res.results, orders)
